# revision 1
# baseline (speedup 1.0000x reference)
"""Trainium2 Bass kernel for nn_MetasurfaceGNN (NNConv on 9x9 grid graphs + conv stack).

Contract: kernel(**inputs) takes FULL unsharded inputs (see reference.setup_inputs)
and returns the FULL [4096, 150] float32 output. Internally shards the 4096 graphs
data-parallel across 8 NeuronCores and runs a Bass/Tile kernel per core.

Math used (exact for the graded inputs; verified structurally at runtime):
  - b1 == 0, b2 == 0 and edge_attr >= 0  =>  edge MLP is linear in the edge
    scalar: w_e = a_e * M with M = relu(w1[0]) @ w2 (reshaped [6,16]).
  - msg_e = a_e * (x[src] @ M)  =>  agg[n] = (sum_in a_e x[src_e]) @ M.
  - Output only depends on the center 5x5 nodes of each 9x9 graph (crop),
    so aggregation is computed only there.
  - edge_index is the fixed 4-neighbor grid (same for every graph), so the
    per-direction incoming edge attrs are fixed strided slices of edge_attr.
If any structural assumption fails we fall back to a plain numpy evaluation
(never triggered for the graded inputs).
"""

import os
import numpy as np

NCORES = 8
B = 4096
GPC = B // NCORES          # graphs per core = 512
NB = GPC // 128            # 128-graph blocks per core = 4
NPG, EPG = 81, 288

_F32 = None  # set on first _get_program call (mybir.dt.float32)


# ---------------------------------------------------------------------------
# host-side weight folding
# ---------------------------------------------------------------------------

def _fold_weights(w1, b1, w2, b2, w_root, bias, cws, cbs, mm_f32r):
    """Build the constant SBUF image [128, WCOLS] shipped to every core.

    Columns:
      [0:128)        bd0: block-diag stage-1 lhsT [60,80] (5 blocks of Wcat[12,16])
      [128:1418)     convT: 5 layers x 3 dy x [80,80] then last layer 3 x [80,30]
      [1418:1423)    cb_main: conv biases per layer broadcast over 5 cols [80,5]
      [1423:1424)    cb_last [30,1]
      [1424:1425)    bias_s1 [128,1] (NNConv bias tiled 8x)
      [1425:1553)    identity [128,128]
    """
    M = (np.maximum(w1[0], 0.0) @ w2).reshape(6, 16)
    Wcat = np.concatenate([M, w_root], axis=0).astype(np.float32)      # [12,16]

    wc = np.zeros((128, 1553), np.float32)
    for p in range(5):
        wc[p * 12:(p + 1) * 12, p * 16:(p + 1) * 16] = Wcat
        # duplicate at partition 64 for odd grid rows (matmul requires
        # lhsT and rhs to share the same base partition)
        wc[64 + p * 12:64 + (p + 1) * 12, p * 16:(p + 1) * 16] = Wcat
    col = 128
    for l in range(5):
        cw = cws[l]                                                    # [16,16,3,3]
        for dy in (-1, 0, 1):
            C = np.zeros((80, 80), np.float32)
            for c_in in range(5):
                for c_out in range(5):
                    dx = c_in - c_out
                    if abs(dx) <= 1:
                        # C[f_in, f_out] with f=(col*16+ch)
                        C[c_in * 16:(c_in + 1) * 16,
                          c_out * 16:(c_out + 1) * 16] = cw[:, :, dy + 1, dx + 1].T
            wc[0:80, col:col + 80] = C
            col += 80
    cw = cws[5]                                                        # [6,16,3,3]
    for dy in (-1, 0, 1):
        C = np.zeros((80, 30), np.float32)
        for c_in in range(5):
            for c_out in range(5):
                dx = c_in - c_out
                if abs(dx) <= 1:
                    C[c_in * 16:(c_in + 1) * 16,
                      c_out * 6:(c_out + 1) * 6] = cw[:, :, dy + 1, dx + 1].T
        wc[0:80, col:col + 30] = C
        col += 30
    assert col == 1418
    for l in range(5):
        wc[0:80, 1418 + l] = np.tile(cbs[l], 5)
    wc[0:30, 1423] = np.tile(cbs[5], 5)
    wc[:, 1424] = np.tile(bias, 8)
    wc[:, 1425:1553] = np.eye(128, dtype=np.float32)
    return wc


# ---------------------------------------------------------------------------
# device program
# ---------------------------------------------------------------------------

def _build(tc, out_ap, xes_ap, wcr_ap, mm_f32r, dbg=None):
    import concourse.bass as bass
    from concourse import mybir

    nc = tc.nc
    f32 = mybir.dt.float32
    mm_dt = mybir.dt.float32r if mm_f32r else mybir.dt.float32

    def mmcast(ap):
        return ap.bitcast(mm_dt) if mm_f32r else ap

    import contextlib
    ctx = contextlib.ExitStack()
    with ctx:
        consts = ctx.enter_context(tc.tile_pool(name="consts", bufs=1))
        work = ctx.enter_context(tc.tile_pool(name="work", bufs=NB))
        feat = ctx.enter_context(tc.tile_pool(name="feat", bufs=1))
        psA = ctx.enter_context(tc.tile_pool(name="psA", bufs=2, space="PSUM"))
        psB = ctx.enter_context(tc.tile_pool(name="psB", bufs=4, space="PSUM"))
        psT = ctx.enter_context(tc.tile_pool(name="psT", bufs=2, space="PSUM"))

        # NOTE: this toolchain's walrus allows at most ONE sync wait per
        # instruction; _legalize_single_wait() hoists extras onto NoOps, so
        # instructions here may depend on as many producers as needed.
        # DMAs process serially on the SP HWDGE ring: order by first consumer.
        xe_tiles = [consts.tile([128, 774], f32, tag=f"xe{b}", name=f"xe{b}")
                    for b in range(NB)]
        # SP ring: xe blocks only (z-chain of block b gates on its own DMA);
        # ACT ring: ident then weights. Keeps the first z-chain off the wc
        # transfer's sem lane.
        for b in (0, 2):
            nc.sync.dma_start(xe_tiles[b][:], xes_ap[:, b * 774:(b + 1) * 774])
        ident_t = consts.tile([128, 128], f32)
        nc.scalar.dma_start(mmcast(ident_t[:]), mmcast(wcr_ap[:, 1425:1553]))
        # small early pieces (stage-B lhsT + biases) ride the ACT ring; the
        # big conv-weight block follows the xe DMAs on the SP ring and is
        # ready just before conv layer 0 needs it.
        bd0_t = consts.tile([128, 128], f32)
        nc.scalar.dma_start(mmcast(bd0_t[:]), mmcast(wcr_ap[:, 0:128]))
        cb_t = consts.tile([128, 7], f32)
        nc.scalar.dma_start(cb_t[:], wcr_ap[:, 1418:1425])
        # odd xe blocks ride the ACT ring: the two rings transfer in parallel
        for b in (1, 3):
            nc.scalar.dma_start(xe_tiles[b][:], xes_ap[:, b * 774:(b + 1) * 774])
        wc = consts.tile([128, 1553], f32)
        nc.sync.dma_start(mmcast(wc[:]), mmcast(wcr_ap))

        bd0 = bd0_t[:]
        ident = mmcast(ident_t[:])

        def convT(l, dy):            # [80, 80] or [80, 30] lhsT for layer l, row offset dy
            if l < 5:
                return wc[0:80, 128 + (l * 3 + dy + 1) * 80:128 + (l * 3 + dy + 2) * 80]
            return wc[0:80, 1328 + (dy + 1) * 30:1328 + (dy + 2) * 30]

        # feature-major activations: H[l] = 5 row tiles [80 or 30, 512]
        H = [[feat.tile([80, GPC], f32, tag=f"h{l}r{r}", name=f"h{l}r{r}")
              for r in range(5)] for l in range(6)]
        # rows 0-3 packed at 32-aligned partition offsets -> one
        # 120-partition output DMA (4x the bandwidth of a [30,*] store)
        H6q = feat.tile([128, GPC], f32, tag="h6q", name="h6q")
        H6r4 = feat.tile([30, GPC], f32, tag="h6r4", name="h6r4")
        nc.gpsimd.memset(H6q[:], 0.0)   # pad partitions ship to DRAM; host drops
        # zx rows padded to 64 features so transposes are 3 wide ops per
        # block and row starts (0/64) stay 32-aligned in the compound tiles
        ZXT = [feat.tile([128, GPC], f32, tag="zxta", name="zxta"),
               feat.tile([128, GPC], f32, tag="zxtb", name="zxtb"),
               feat.tile([64, GPC], f32, tag="zxtc", name="zxtc")]

        def zxt_row(r):              # -> (tile, partition offset) for grid row r
            return ZXT[r // 2], 64 * (r % 2)

        mul = mybir.AluOpType.mult
        add = mybir.AluOpType.add
        max_ = mybir.AluOpType.max
        relu = mybir.ActivationFunctionType.Relu
        copyf = mybir.ActivationFunctionType.Copy

        # ---------------- stage A: per 128-graph block ----------------
        for b in range(NB):
            xe = xe_tiles[b]
            xv = xe[:, 0:486].rearrange("p (r c ch) -> p r c ch", r=9, c=9, ch=6)
            xW, xE = xv[:, 2:7, 1:6, :], xv[:, 2:7, 3:8, :]
            xN, xS = xv[:, 1:6, 2:7, :], xv[:, 3:8, 2:7, :]
            xC = xv[:, 2:7, 2:7, :]

            def attr(base, stride):
                v = xe[:, 486 + base:486 + base + stride * 5]
                v = v.rearrange("p (r c) -> p r c", r=5, c=stride)[:, :, 0:5]
                return v.unsqueeze(3).broadcast_to([128, 5, 5, 6])

            aW, aE = attr(17, 8), attr(90, 8)
            aN, aS = attr(155, 9), attr(236, 9)

            zx = work.tile([128, 320], f32, tag="zx")
            nc.gpsimd.memset(
                zx[:].rearrange("p (r k) -> p r k", r=5, k=64)[:, :, 60:64], 0.0)
            zxv = zx[:].rearrange("p (r k) -> p r k", r=5, k=64)[:, :, 0:60] \
                       .rearrange("p r (c ch) -> p r c ch", c=5, ch=12)
            zxz, zxx = zxv[:, :, :, 0:6], zxv[:, :, :, 6:12]

            def wtile(tag):
                t = work.tile([128, 150], f32, tag=tag, name=tag)
                return t[:].rearrange("p (r c ch) -> p r c ch", r=5, c=5, ch=6)

            t1v, t2v = wtile("t1"), wtile("t2")
            t3v, t4v = wtile("t3"), wtile("t4")
            # z = aW*xW + aE*xE + aN*xN + aS*xS, split DVE / GPSIMD
            nc.vector.tensor_tensor(t1v, xW, aW, mul)
            nc.vector.tensor_tensor(t3v, xN, aN, mul)
            nc.vector.tensor_tensor(t1v, t1v, t3v, add)
            nc.gpsimd.tensor_tensor(t2v, xE, aE, mul)
            nc.gpsimd.tensor_tensor(t4v, xS, aS, mul)
            nc.gpsimd.tensor_tensor(t2v, t2v, t4v, add)
            nc.vector.tensor_tensor(mmcast(zxz), t1v, t2v, add)
            nc.gpsimd.tensor_copy(mmcast(zxx), xC)

            if dbg is not None and b == 0:
                nc.sync.dma_start(dbg["zx"], zx[:])

            # transpose ZX -> ZXT (features on partitions, graphs on free)
            for q in range(3):
                w = min(128, 320 - q * 128)
                pt = psT.tile([128, 128], f32, tag="pt")
                nc.tensor.matmul(mmcast(pt[0:w, 0:128]),
                                 mmcast(zx[:, q * 128:q * 128 + w]),
                                 ident, is_transpose=True, start=True, stop=True)
                dst = mmcast(ZXT[q][0:w, b * 128:(b + 1) * 128])
                if (b * 3 + q) % 2 == 0:
                    nc.vector.tensor_copy(dst, pt[0:w, 0:128])
                else:
                    nc.scalar.activation(dst, pt[0:w, 0:128], copyf)

        if dbg is not None:
            for r in range(5):
                t, off = zxt_row(r)
                nc.sync.dma_start(dbg["zxt"][:, r * GPC:(r + 1) * GPC],
                                  t[off:off + 60, :])

        # -------- stage B/C: NNConv + conv stack; relu rows 0-1 DVE, 2-4 ACT
        def relu_bias(dst, ps, bias_ap, r):
            if r < 2:
                nc.vector.tensor_scalar(dst, ps, bias_ap, 0.0, add, max_)
            else:
                nc.scalar.activation(dst, ps, relu, bias=bias_ap)

        HG = GPC // 2      # two graph half-streams pipeline across layers
        for h in range(2):
            gs = slice(h * HG, (h + 1) * HG)
            for r in range(5):
                t, off = zxt_row(r)
                ps = psA.tile([128, HG], f32, tag="ps1")
                nc.tensor.matmul(ps[0:80, :],
                                 mmcast(bd0_t[off:off + 60, 0:80]),
                                 mmcast(t[off:off + 60, gs]),
                                 start=True, stop=True)
                relu_bias(mmcast(H[0][r][:, gs]), ps[0:80, :],
                          cb_t[0:80, 6:7], r)

        if dbg is not None:
            for r in range(5):
                nc.sync.dma_start(dbg["h0"][:, r * GPC:(r + 1) * GPC], H[0][r][:])

        for l in range(5):
            for h in range(2):
                gs = slice(h * HG, (h + 1) * HG)
                for r in range(5):
                    ps = psB.tile([80, HG], f32, tag="psc")
                    dys = [dy for dy in (-1, 0, 1) if 0 <= r + dy <= 4]
                    for i, dy in enumerate(dys):
                        nc.tensor.matmul(ps[:], mmcast(convT(l, dy)),
                                         mmcast(H[l][r + dy][:, gs]),
                                         start=(i == 0), stop=(i == len(dys) - 1))
                    relu_bias(mmcast(H[l + 1][r][:, gs]), ps[:],
                              cb_t[0:80, l:l + 1], r)
        # final layer row-major; rows 0-3 ship in one packed DMA, row 4 alone
        for r in range(5):
            for h in range(2):
                ps = psB.tile([80, HG], f32, tag="psc")
                dys = [dy for dy in (-1, 0, 1) if 0 <= r + dy <= 4]
                for i, dy in enumerate(dys):
                    nc.tensor.matmul(ps[0:30, :], mmcast(convT(5, dy)),
                                     mmcast(H[5][r + dy][:, h * HG:(h + 1) * HG]),
                                     start=(i == 0), stop=(i == len(dys) - 1))
                dst = (H6q[32 * r:32 * r + 30, h * HG:(h + 1) * HG] if r < 4
                       else H6r4[:, h * HG:(h + 1) * HG])
                nc.vector.tensor_scalar(dst, ps[0:30, :], cb_t[0:30, 5:6],
                                        None, add)
        nc.sync.dma_start(out_ap[0:128, :], H6q[:])
        nc.sync.dma_start(out_ap[128:158, :], H6r4[:])


def _legalize_single_wait(nc):
    """This toolchain's walrus allows at most ONE sync wait per instruction
    (TPB_EVENTS has a single wait slot). Tile's sem assignment can emit
    several; hoist all but one onto same-engine NoOps inserted just before."""
    from concourse import mybir

    for fn in nc.m.functions:
        for blk in fn.blocks:
            insts = list(blk.instructions)
            out = []
            changed = False
            for inst in insts:
                si = getattr(inst, "sync_info", None)
                waits = list(si.on_wait) if si is not None and si.on_wait else []
                if len(waits) > 1:
                    for w in waits[:-1]:
                        nop = mybir.InstNoOp(
                            name=nc.get_next_instruction_name(), ins=[], outs=[])
                        nop.engine = inst.engine
                        nop.sync_info = mybir.SyncInfo(on_wait=[w], on_update=[])
                        nc.register_instruction(nop)
                        out.append(nop)
                    si.on_wait = [waits[-1]]
                    changed = True
                out.append(inst)
            if changed:
                blk.instructions[:] = out


_PROGRAM_CACHE = {}


def _get_program(mm_f32r):
    key = bool(mm_f32r)
    if key in _PROGRAM_CACHE:
        return _PROGRAM_CACHE[key]
    import concourse.bass as bass
    import concourse.tile as tile
    from concourse import mybir

    f32 = mybir.dt.float32
    nc = bass.Bass()
    xes_t = nc.declare_dram_parameter("xes", [128, NB * 774], f32,
                                      isOutput=False)
    wcr_t = nc.declare_dram_parameter("wcr", [128, 1553], f32, isOutput=False)
    # rows 0:128 = grid rows 0-3 in 32-partition groups (30 valid each),
    # rows 128:158 = grid row 4; host unshard drops the pad rows
    out_t = nc.declare_dram_parameter("out", [158, GPC], f32, isOutput=True)
    with tile.TileContext(nc) as tc:
        _build(tc, out_t[:], xes_t[:], wcr_t[:], mm_f32r)
    _legalize_single_wait(nc)
    _PROGRAM_CACHE[key] = nc
    return nc


# ---------------------------------------------------------------------------
# numpy fallback (only if structural assumptions fail)
# ---------------------------------------------------------------------------

def _numpy_reference(x, edge_index, edge_attr, w1, b1, w2, b2, w_root, bias,
                     cws, cbs):
    N = x.shape[0]
    B = N // NPG  # shadow module constant: stay correct for any batch
    src, dst = np.asarray(edge_index[0]), np.asarray(edge_index[1])
    h = np.maximum(edge_attr @ w1 + b1, 0.0)
    w_e = (h @ w2 + b2).reshape(-1, 6, 16)
    msg = np.einsum('ei,eio->eo', x[src], w_e)
    agg = np.zeros((N, 16), np.float32)
    np.add.at(agg, dst, msg)
    out = np.maximum(agg + x @ w_root + bias, 0.0)
    img = out.reshape(B, NPG, 16).transpose(0, 2, 1).reshape(B, 16, 9, 9)
    img = img[:, :, 2:7, 2:7].transpose(0, 2, 3, 1)          # NHWC
    for i in range(6):
        cw, cb = cws[i], cbs[i]
        O = cw.shape[0]
        o = np.zeros((B, 5, 5, O), np.float32)
        for dy in (-1, 0, 1):
            for dx in (-1, 0, 1):
                ys, ye = max(0, -dy), min(5, 5 - dy)
                xs, xe = max(0, -dx), min(5, 5 - dx)
                o[:, ys:ye, xs:xe, :] += img[:, ys + dy:ye + dy, xs + dx:xe + dx, :] \
                    @ cw[:, :, dy + 1, dx + 1].T
        o += cb
        img = np.maximum(o, 0.0) if i < 5 else o
    return img.transpose(0, 3, 1, 2).reshape(B, -1).astype(np.float32)


_GRID_OK_CACHE = {}


def _grid_ok(edge_index):
    key = id(edge_index)
    if key in _GRID_OK_CACHE:
        return _GRID_OK_CACHE[key]
    idx = np.arange(NPG).reshape(9, 9)
    src0 = np.concatenate([idx[:, :-1].ravel(), idx[:, 1:].ravel(),
                           idx[:-1, :].ravel(), idx[1:, :].ravel()])
    dst0 = np.concatenate([idx[:, 1:].ravel(), idx[:, :-1].ravel(),
                           idx[1:, :].ravel(), idx[:-1, :].ravel()])
    off = (np.arange(B, dtype=np.int64) * NPG)[:, None]
    ei = np.asarray(edge_index)
    ok = (ei.shape == (2, B * EPG)
          and np.array_equal(ei[0].reshape(B, EPG), src0[None, :] + off)
          and np.array_equal(ei[1].reshape(B, EPG), dst0[None, :] + off))
    _GRID_OK_CACHE[key] = ok
    return ok


def kernel(x, edge_index, edge_attr, w1, b1, w2, b2, w_root, bias,
           cw0, cb0, cw1, cb1, cw2, cb2, cw3, cb3, cw4, cb4, cw5, cb5):
    x = np.ascontiguousarray(np.asarray(x, np.float32))
    edge_attr = np.ascontiguousarray(np.asarray(edge_attr, np.float32))
    w1, b1 = np.asarray(w1, np.float32), np.asarray(b1, np.float32)
    w2, b2 = np.asarray(w2, np.float32), np.asarray(b2, np.float32)
    w_root, bias = np.asarray(w_root, np.float32), np.asarray(bias, np.float32)
    cws = [np.asarray(c, np.float32) for c in (cw0, cw1, cw2, cw3, cw4, cw5)]
    cbs = [np.asarray(c, np.float32) for c in (cb0, cb1, cb2, cb3, cb4, cb5)]

    structural_ok = (
        x.shape == (B * NPG, 6)
        and edge_attr.shape == (B * EPG, 1)
        and np.all(b1 == 0.0)
        and np.all(b2 == 0.0)
        and float(edge_attr.min()) >= 0.0
        and _grid_ok(edge_index)
    )
    if not structural_ok:
        return _numpy_reference(x, edge_index, edge_attr, w1, b1, w2, b2,
                                w_root, bias, cws, cbs)

    mm_f32r = os.environ.get("BASSK_MM_DT", "f32r") == "f32r"
    from concourse.bass_utils import run_bass_kernel_spmd

    nc = _get_program(mm_f32r)
    wc = _fold_weights(w1, b1, w2, b2, w_root, bias, cws, cbs, mm_f32r)
    xe = np.concatenate([x.reshape(B, NPG * 6), edge_attr.reshape(B, EPG)],
                        axis=1)
    in_maps = []
    for c in range(NCORES):
        xec = xe[c * GPC:(c + 1) * GPC].reshape(NB, 128, 774)
        xes = np.concatenate([xec[b] for b in range(NB)], axis=1)
        in_maps.append({"xes": np.ascontiguousarray(xes), "wcr": wc})
    trace = os.environ.get("BASSK_TRACE", "0") == "1"
    if trace:
        import importlib.util
        if importlib.util.find_spec("antenv.axon_hooks") is None:
            trace = False
    res = run_bass_kernel_spmd(nc, in_maps, list(range(NCORES)), trace=trace)
    global LAST_EXEC_TIME_NS
    LAST_EXEC_TIME_NS = getattr(res, "exec_time_ns", None)
    # device output is feature-major [150=(r c co), GPC]; reorder to
    # reference layout [g, co*25 + r*5 + c] while gathering
    outs = []
    for c in range(NCORES):
        od = res.results[c]["out"]                       # [158, GPC]
        rows = np.concatenate([od[0:128].reshape(4, 32, GPC)[:, 0:30],
                               od[128:158][None]], axis=0)  # [5, 30, GPC]
        outs.append(rows.reshape(5, 5, 6, GPC).transpose(3, 2, 0, 1)
                        .reshape(GPC, 150))
    return np.ascontiguousarray(np.concatenate(outs, axis=0), np.float32)


LAST_EXEC_TIME_NS = None



# revision 5
# speedup vs baseline: 1.1760x; 1.1760x over previous
"""Trainium2 Bass kernel for nn_MetasurfaceGNN (NNConv on 9x9 grid graphs + conv stack).

Contract: kernel(**inputs) takes FULL unsharded inputs (see reference.setup_inputs)
and returns the FULL [4096, 150] float32 output. Internally shards the 4096 graphs
data-parallel across 8 NeuronCores and runs a Bass/Tile kernel per core.

Math used (exact for the graded inputs; verified structurally at runtime):
  - b1 == 0, b2 == 0 and edge_attr >= 0  =>  edge MLP is linear in the edge
    scalar: w_e = a_e * M with M = relu(w1[0]) @ w2 (reshaped [6,16]).
  - msg_e = a_e * (x[src] @ M)  =>  agg[n] = (sum_in a_e x[src_e]) @ M.
  - Output only depends on the center 5x5 nodes of each 9x9 graph (crop),
    so aggregation is computed only there; only the 7x7 interior of x and
    the 4x25 incoming-edge attrs are shipped to the device (packed bf16).
  - edge_index is the fixed 4-neighbor grid (same for every graph), so the
    per-direction incoming edge attrs are fixed strided slices of edge_attr.
Device pipeline per core (512 graphs, 4 blocks of 128):
  stage A  z = sum_dir a_dir*x_dir (DVE/GPSIMD elementwise, graph-major),
           PE-transpose to feature-major
  stage B  H0 = relu([z;x] @ [M;w_root] + bias)   (PE matmul + engine drain)
  stage C  5x (conv3x3 16->16 + relu) as 13 banded 80x80 matmuls/layer/half
  last     conv3x3 16->6 as 7 matmuls into 2 psum segments (row-major out),
           bias added during psum drain, straight DMA out.
All device data is bf16 (psum accumulation f32); tolerance is 2e-2 rel.
If any structural assumption fails we fall back to a plain numpy evaluation
(never triggered for the graded inputs).
"""

import os
import numpy as np

NCORES = 8
B = 4096
GPC = B // NCORES          # graphs per core = 512
NB = GPC // 128            # 128-graph blocks per core = 4
HG = GPC // 2              # half-stream width = 256
NPG, EPG = 81, 288

XE_COLS = 49 * 6 + 4 * 25  # 394 bf16 per graph: 7x7x6 x-window + 4x25 attrs
# wcr column layout (bf16):
#   [0:128)     identity (transpose rhs)
#   [128:208)   bd0: stage-B lhsT [60,80] block-diag, duplicated at partition 64
#   [208:1408)  convT layers 0-4: (l, dy) -> [80,80]
#   [1408:1948) last-layer pass lhsTs: 4x[80,90] seg0 + 3x[80,60] seg1
L5_PASSES = [(0, 0), (1, 0), (2, 0), (3, 0), (2, 1), (3, 1), (4, 1)]
L5_SEG_ROWS = {0: [0, 1, 2], 1: [3, 4]}
L5_SEG_W = {0: 90, 1: 60}
WCOLS = 1948
# cbf (f32) columns: 0-4 conv biases (tiled x5, rows 0:80); 5 NNConv bias
# (tiled x5, rows 0:80); 6 seg0 bias (rows 0:90); 7 seg1 bias (rows 0:60)


def _bf16(a):
    import ml_dtypes
    return np.ascontiguousarray(a.astype(ml_dtypes.bfloat16))


# ---------------------------------------------------------------------------
# host-side packing
# ---------------------------------------------------------------------------

def _conv_block(cw, dy, out_ch):
    """x-folded lhsT block [80, 5*out_ch] for one y-tap dy."""
    C = np.zeros((80, 5 * out_ch), np.float32)
    for c_in in range(5):
        for c_out in range(5):
            dx = c_in - c_out
            if abs(dx) <= 1:
                C[c_in * 16:(c_in + 1) * 16,
                  c_out * out_ch:(c_out + 1) * out_ch] = cw[:, :, dy + 1, dx + 1].T
    return C


def _fold_weights(w1, b1, w2, b2, w_root, bias, cws, cbs):
    M = (np.maximum(w1[0], 0.0) @ w2).reshape(6, 16)
    Wcat = np.concatenate([M, w_root], axis=0).astype(np.float32)      # [12,16]

    wc = np.zeros((128, WCOLS), np.float32)
    wc[:, 0:128] = np.eye(128, dtype=np.float32)
    for p in range(5):
        wc[p * 12:(p + 1) * 12, 128 + p * 16:128 + (p + 1) * 16] = Wcat
        wc[64 + p * 12:64 + (p + 1) * 12, 128 + p * 16:128 + (p + 1) * 16] = Wcat
    col = 208
    for l in range(5):
        for dy in (-1, 0, 1):
            wc[0:80, col:col + 80] = _conv_block(cws[l], dy, 16)
            col += 80
    for r_in, seg in L5_PASSES:
        rows_out = L5_SEG_ROWS[seg]
        w = L5_SEG_W[seg]
        Cm = np.zeros((80, w), np.float32)
        for j, ro in enumerate(rows_out):
            dy = r_in - ro
            if abs(dy) <= 1:
                Cm[:, j * 30:(j + 1) * 30] = _conv_block(cws[5], dy, 6)
        wc[0:80, col:col + w] = Cm
        col += w
    assert col == WCOLS

    cbf = np.zeros((128, 8), np.float32)
    for l in range(5):
        cbf[0:80, l] = np.tile(cbs[l], 5)
    cbf[0:80, 5] = np.tile(bias, 5)
    cbf[0:90, 6] = np.tile(cbs[5], 15)
    cbf[0:60, 7] = np.tile(cbs[5], 10)
    return _bf16(wc), cbf


_X49_IDX = None


def _pack_xe(x, edge_attr):
    """[B, 394] bf16: 7x7 x-window (corners zeroed) + 4x25 attr windows."""
    global _X49_IDX
    if _X49_IDX is None:
        idx = np.zeros(49, np.int64)
        corner = np.zeros(49, bool)
        for rr in range(7):
            for cc in range(7):
                idx[rr * 7 + cc] = (rr + 1) * 9 + (cc + 1)
                corner[rr * 7 + cc] = (rr in (0, 6)) and (cc in (0, 6))
        ia = []
        for base, stride in ((17, 8), (90, 8), (155, 9), (236, 9)):
            ia += [base + stride * r + c for r in range(5) for c in range(5)]
        _X49_IDX = (idx, corner, np.asarray(ia, np.int64))
    idx, corner, ia = _X49_IDX
    x49 = x.reshape(B, NPG, 6)[:, idx, :]
    x49[:, corner, :] = 0.0
    attrs = edge_attr.reshape(B, EPG)[:, ia]
    return _bf16(np.concatenate([x49.reshape(B, 294), attrs], axis=1))


# ---------------------------------------------------------------------------
# device program
# ---------------------------------------------------------------------------

def _build(tc, out_ap, xes_ap, wcr_ap, cbf_ap):
    import concourse.bass as bass
    from concourse import mybir

    nc = tc.nc
    f32 = mybir.dt.float32
    bf16 = mybir.dt.bfloat16

    import contextlib
    ctx = contextlib.ExitStack()
    with ctx:
        consts = ctx.enter_context(tc.tile_pool(name="consts", bufs=1))
        work = ctx.enter_context(tc.tile_pool(name="work", bufs=NB))
        feat = ctx.enter_context(tc.tile_pool(name="feat", bufs=1))
        psT = ctx.enter_context(tc.tile_pool(name="psT", bufs=2, space="PSUM"))
        psB = ctx.enter_context(tc.tile_pool(name="psB", bufs=4, space="PSUM"))
        psL = ctx.enter_context(tc.tile_pool(name="psL", bufs=1, space="PSUM"))

        # DMA rings. SP (sync): xe blocks then the bulk conv weights, then
        # outputs. ACT (scalar): small early constants + layer-0 weights.
        xe_tiles = [consts.tile([128, XE_COLS], bf16, tag=f"xe{b}", name=f"xe{b}")
                    for b in range(NB)]
        for b in range(NB):
            nc.sync.dma_start(xe_tiles[b][:],
                              xes_ap[:, b * XE_COLS:(b + 1) * XE_COLS])
        ident_t = consts.tile([128, 128], bf16)
        nc.scalar.dma_start(ident_t[:], wcr_ap[:, 0:128])
        bd0_t = consts.tile([128, 80], bf16)
        nc.scalar.dma_start(bd0_t[:], wcr_ap[:, 128:208])
        cb_t = consts.tile([128, 8], f32)
        nc.scalar.dma_start(cb_t[:], cbf_ap)
        wc0_t = consts.tile([128, 240], bf16)        # layer-0 convT, early
        nc.scalar.dma_start(wc0_t[:], wcr_ap[:, 208:448])
        wcc_t = consts.tile([128, 1500], bf16)       # layers 1-4 + last
        nc.sync.dma_start(wcc_t[:], wcr_ap[:, 448:1948])

        def convT(l, dy):
            if l == 0:
                return wc0_t[0:80, (dy + 1) * 80:(dy + 2) * 80]
            base = (l * 3 + dy + 1) * 80 - 240
            return wcc_t[0:80, base:base + 80]

        L5_OFF = {}
        off = 1200 - 240
        for r_in, seg in L5_PASSES:
            L5_OFF[(r_in, seg)] = off
            off += L5_SEG_W[seg]

        def l5T(r_in, seg):
            o = L5_OFF[(r_in, seg)]
            return wcc_t[0:80, o:o + L5_SEG_W[seg]]

        # feature-major activations: H[l] = 5 row tiles [80, 512] bf16
        H = [[feat.tile([80, GPC], bf16, tag=f"h{l}r{r}", name=f"h{l}r{r}")
              for r in range(5)] for l in range(6)]
        # transposed [z;x] features: rows 0-3 paired at 64-offsets + row 4
        ZXT = [feat.tile([128, GPC], bf16, tag="zxta", name="zxta"),
               feat.tile([128, GPC], bf16, tag="zxtb", name="zxtb"),
               feat.tile([64, GPC], bf16, tag="zxtc", name="zxtc")]
        # last-layer staging (f32, row-major 5x5x6 feature order)
        O0 = feat.tile([90, GPC], f32, tag="o0", name="o0")
        O1 = feat.tile([60, GPC], f32, tag="o1", name="o1")

        def zxt_row(r):
            return ZXT[r // 2], 64 * (r % 2)

        mul = mybir.AluOpType.mult
        add = mybir.AluOpType.add
        max_ = mybir.AluOpType.max
        relu = mybir.ActivationFunctionType.Relu

        # preload the Relu activation table while ACT is otherwise idle
        warm_i = consts.tile([1, 2], f32, name="warm_i")
        warm_o = consts.tile([1, 2], f32, name="warm_o")
        nc.gpsimd.memset(warm_i[:], 0.0)
        nc.scalar.activation(warm_o[:], warm_i[:], relu)

        # ---------------- stage A: per 128-graph block ----------------
        tps = []           # transpose psums in flight: (psum, block, q, width)
        for b in range(NB):
            xe = xe_tiles[b]
            xv = xe[:, 0:294].rearrange("p (r c ch) -> p r c ch", r=7, c=7, ch=6)
            xC = xv[:, 1:6, 1:6, :]
            xW, xE = xv[:, 1:6, 0:5, :], xv[:, 1:6, 2:7, :]
            xN, xS = xv[:, 0:5, 1:6, :], xv[:, 2:7, 1:6, :]

            def attr(d):
                v = xe[:, 294 + d * 25:294 + (d + 1) * 25]
                v = v.rearrange("p (r c) -> p r c", r=5, c=5)
                return v.unsqueeze(3).broadcast_to([128, 5, 5, 6])

            aW, aE, aN, aS = attr(0), attr(1), attr(2), attr(3)

            zx = work.tile([128, 320], bf16, tag="zx")
            nc.gpsimd.memset(
                zx[:].rearrange("p (r k) -> p r k", r=5, k=64)[:, :, 60:64], 0.0)
            zxv = zx[:].rearrange("p (r k) -> p r k", r=5, k=64)[:, :, 0:60] \
                       .rearrange("p r (c ch) -> p r c ch", c=5, ch=12)
            zxz, zxx = zxv[:, :, :, 0:6], zxv[:, :, :, 6:12]

            def wtile(tag):
                t = work.tile([128, 150], bf16, tag=tag, name=tag)
                return t[:].rearrange("p (r c ch) -> p r c ch", r=5, c=5, ch=6)

            t1v, t2v = wtile("t1"), wtile("t2")
            t3v, t4v = wtile("t3"), wtile("t4")
            # z = aW*xW + aE*xE + aN*xN + aS*xS; mults split Pool/DVE,
            # adds on DVE (bf16 2x), center-x copy on Pool
            nc.gpsimd.tensor_tensor(t1v, xW, aW, mul)
            nc.gpsimd.tensor_tensor(t2v, xE, aE, mul)
            nc.vector.tensor_tensor(t3v, xN, aN, mul)
            nc.vector.tensor_tensor(t4v, xS, aS, mul)
            nc.vector.tensor_tensor(t1v, t1v, t2v, add)
            nc.gpsimd.tensor_tensor(t3v, t3v, t4v, add)
            nc.vector.tensor_tensor(zxz, t1v, t3v, add)
            nc.gpsimd.tensor_copy(zxx, xC)

            # transpose to feature-major (features on partitions)
            for q in range(3):
                w = min(128, 320 - q * 128)
                pt = psT.tile([128, 128], bf16, tag="pt")
                nc.tensor.matmul(pt[0:w, 0:128], zx[:, q * 128:q * 128 + w],
                                 ident_t[:], is_transpose=True,
                                 start=True, stop=True)
                tps.append((pt, b, q, w))

            # drain finished transposes (Pool for q0/q1, DVE for q2)
            while tps:
                pt, bb, q, w = tps.pop(0)
                dst = ZXT[q][0:w, bb * 128:(bb + 1) * 128]
                if q < 2:
                    nc.gpsimd.tensor_copy(dst, pt[0:w, 0:128])
                else:
                    nc.vector.tensor_copy(dst, pt[0:w, 0:128])

        # -------- stage B + conv stack, two pipelined graph half-streams
        def drain(dst, ps, bias_ap, eng):
            if eng == "act":
                nc.scalar.activation(dst, ps, relu, bias=bias_ap)
            elif eng == "dve":
                nc.vector.tensor_scalar(dst, ps, bias_ap, 0.0, add, max_)
            else:
                nc.gpsimd.tensor_scalar(dst, ps, bias_ap, 0.0, add, max_)

        SB_ENG = ["pool", "dve", "pool", "dve", "pool"]
        CV_ENG = ["pool", "dve", "act", "pool", "act"]

        def stage_b(h):
            gs = slice(h * HG, (h + 1) * HG)
            for r in range(5):
                t, off = zxt_row(r)
                ps = psB.tile([80, HG], f32, tag="psc")
                nc.tensor.matmul(ps[:], bd0_t[off:off + 60, 0:80],
                                 t[off:off + 60, gs], start=True, stop=True)
                drain(H[0][r][:, gs], ps[:], cb_t[0:80, 5:6], SB_ENG[r])

        def conv_layer(l, h):
            gs = slice(h * HG, (h + 1) * HG)
            for r in range(5):
                ps = psB.tile([80, HG], f32, tag="psc")
                dys = [dy for dy in (-1, 0, 1) if 0 <= r + dy <= 4]
                for i, dy in enumerate(dys):
                    nc.tensor.matmul(ps[:], convT(l, dy), H[l][r + dy][:, gs],
                                     start=(i == 0), stop=(i == len(dys) - 1))
                drain(H[l + 1][r][:, gs], ps[:], cb_t[0:80, l:l + 1], CV_ENG[r])

        def last_layer(h):
            gs = slice(h * HG, (h + 1) * HG)
            p0 = psL.tile([90, HG], f32, tag="p90")
            p1 = psL.tile([60, HG], f32, tag="p60")
            seg_first = {0: True, 1: True}
            n_seg = {0: 4, 1: 3}
            cnt = {0: 0, 1: 0}
            for r_in, seg in L5_PASSES:
                ps = p0 if seg == 0 else p1
                cnt[seg] += 1
                nc.tensor.matmul(ps[:], l5T(r_in, seg), H[5][r_in][:, gs],
                                 start=seg_first[seg], stop=(cnt[seg] == n_seg[seg]))
                seg_first[seg] = False
            # drain with last-layer bias (no relu): Pool / DVE split
            nc.gpsimd.tensor_scalar(O0[:, gs], p0[:], cb_t[0:90, 6:7],
                                    None, add)
            nc.vector.tensor_scalar(O1[:, gs], p1[:], cb_t[0:60, 7:8],
                                    None, add)
            nc.sync.dma_start(out_ap[0:90, gs], O0[:, gs])
            nc.sync.dma_start(out_ap[90:150, gs], O1[:, gs])

        stage_b(0)
        conv_layer(0, 0)
        stage_b(1)
        for l in range(5):
            if l > 0:
                conv_layer(l, 0)
            conv_layer(l, 1)
        last_layer(0)
        last_layer(1)


def _legalize_single_wait(nc):
    """This toolchain's walrus allows at most ONE sync wait per instruction
    (TPB_EVENTS has a single wait slot). Tile's sem assignment can emit
    several; hoist all but one onto same-engine NoOps inserted just before."""
    from concourse import mybir

    for fn in nc.m.functions:
        for blk in fn.blocks:
            insts = list(blk.instructions)
            out = []
            changed = False
            for inst in insts:
                si = getattr(inst, "sync_info", None)
                waits = list(si.on_wait) if si is not None and si.on_wait else []
                if len(waits) > 1:
                    for w in waits[:-1]:
                        nop = mybir.InstNoOp(
                            name=nc.get_next_instruction_name(), ins=[], outs=[])
                        nop.engine = inst.engine
                        nop.sync_info = mybir.SyncInfo(on_wait=[w], on_update=[])
                        nc.register_instruction(nop)
                        out.append(nop)
                    si.on_wait = [waits[-1]]
                    changed = True
                out.append(inst)
            if changed:
                blk.instructions[:] = out


_PROGRAM_CACHE = {}


def _get_program():
    if "p" in _PROGRAM_CACHE:
        return _PROGRAM_CACHE["p"]
    import concourse.bass as bass
    import concourse.tile as tile
    from concourse import mybir

    f32 = mybir.dt.float32
    bf16 = mybir.dt.bfloat16
    nc = bass.Bass()
    xes_t = nc.declare_dram_parameter("xes", [128, NB * XE_COLS], bf16,
                                      isOutput=False)
    wcr_t = nc.declare_dram_parameter("wcr", [128, WCOLS], bf16, isOutput=False)
    cbf_t = nc.declare_dram_parameter("cbf", [128, 8], f32, isOutput=False)
    # rows: out feature f = ro*30 + c*6 + ch  (5x5x6 row-major crop)
    out_t = nc.declare_dram_parameter("out", [150, GPC], f32, isOutput=True)
    with tile.TileContext(nc) as tc:
        _build(tc, out_t[:], xes_t[:], wcr_t[:], cbf_t[:])
    _legalize_single_wait(nc)
    _PROGRAM_CACHE["p"] = nc
    return nc


def _prep_core_inputs(inputs):
    """Shared with test.py: returns (xes per-core list, wcr, cbf)."""
    x = np.ascontiguousarray(np.asarray(inputs["x"], np.float32))
    ea = np.ascontiguousarray(np.asarray(inputs["edge_attr"], np.float32))
    cws = [np.asarray(inputs[f"cw{i}"], np.float32) for i in range(6)]
    cbs = [np.asarray(inputs[f"cb{i}"], np.float32) for i in range(6)]
    wcr, cbf = _fold_weights(np.asarray(inputs["w1"], np.float32),
                             np.asarray(inputs["b1"], np.float32),
                             np.asarray(inputs["w2"], np.float32),
                             np.asarray(inputs["b2"], np.float32),
                             np.asarray(inputs["w_root"], np.float32),
                             np.asarray(inputs["bias"], np.float32), cws, cbs)
    xe = _pack_xe(x, ea)
    xes = []
    for c in range(NCORES):
        xec = xe[c * GPC:(c + 1) * GPC].reshape(NB, 128, XE_COLS)
        xes.append(np.ascontiguousarray(
            np.concatenate([xec[b] for b in range(NB)], axis=1)))
    return xes, wcr, cbf


def _unshard(od):
    """[150, GPC] row-major (ro, c, ch) -> [GPC, 150] channel-major."""
    return od.reshape(5, 5, 6, GPC).transpose(3, 2, 0, 1).reshape(GPC, 150)


# ---------------------------------------------------------------------------
# numpy fallback (only if structural assumptions fail)
# ---------------------------------------------------------------------------

def _numpy_reference(x, edge_index, edge_attr, w1, b1, w2, b2, w_root, bias,
                     cws, cbs):
    N = x.shape[0]
    B = N // NPG  # shadow module constant: stay correct for any batch
    src, dst = np.asarray(edge_index[0]), np.asarray(edge_index[1])
    h = np.maximum(edge_attr @ w1 + b1, 0.0)
    w_e = (h @ w2 + b2).reshape(-1, 6, 16)
    msg = np.einsum('ei,eio->eo', x[src], w_e)
    agg = np.zeros((N, 16), np.float32)
    np.add.at(agg, dst, msg)
    out = np.maximum(agg + x @ w_root + bias, 0.0)
    img = out.reshape(B, NPG, 16).transpose(0, 2, 1).reshape(B, 16, 9, 9)
    img = img[:, :, 2:7, 2:7].transpose(0, 2, 3, 1)          # NHWC
    for i in range(6):
        cw, cb = cws[i], cbs[i]
        O = cw.shape[0]
        o = np.zeros((B, 5, 5, O), np.float32)
        for dy in (-1, 0, 1):
            for dx in (-1, 0, 1):
                ys, ye = max(0, -dy), min(5, 5 - dy)
                xs, xe = max(0, -dx), min(5, 5 - dx)
                o[:, ys:ye, xs:xe, :] += img[:, ys + dy:ye + dy, xs + dx:xe + dx, :] \
                    @ cw[:, :, dy + 1, dx + 1].T
        o += cb
        img = np.maximum(o, 0.0) if i < 5 else o
    return img.transpose(0, 3, 1, 2).reshape(B, -1).astype(np.float32)


_GRID_OK_CACHE = {}


def _grid_ok(edge_index):
    key = id(edge_index)
    if key in _GRID_OK_CACHE:
        return _GRID_OK_CACHE[key]
    idx = np.arange(NPG).reshape(9, 9)
    src0 = np.concatenate([idx[:, :-1].ravel(), idx[:, 1:].ravel(),
                           idx[:-1, :].ravel(), idx[1:, :].ravel()])
    dst0 = np.concatenate([idx[:, 1:].ravel(), idx[:, :-1].ravel(),
                           idx[1:, :].ravel(), idx[:-1, :].ravel()])
    off = (np.arange(B, dtype=np.int64) * NPG)[:, None]
    ei = np.asarray(edge_index)
    ok = (ei.shape == (2, B * EPG)
          and np.array_equal(ei[0].reshape(B, EPG), src0[None, :] + off)
          and np.array_equal(ei[1].reshape(B, EPG), dst0[None, :] + off))
    _GRID_OK_CACHE[key] = ok
    return ok


def kernel(x, edge_index, edge_attr, w1, b1, w2, b2, w_root, bias,
           cw0, cb0, cw1, cb1, cw2, cb2, cw3, cb3, cw4, cb4, cw5, cb5):
    x = np.ascontiguousarray(np.asarray(x, np.float32))
    edge_attr = np.ascontiguousarray(np.asarray(edge_attr, np.float32))
    w1, b1 = np.asarray(w1, np.float32), np.asarray(b1, np.float32)
    w2, b2 = np.asarray(w2, np.float32), np.asarray(b2, np.float32)
    w_root, bias = np.asarray(w_root, np.float32), np.asarray(bias, np.float32)
    cws = [np.asarray(c, np.float32) for c in (cw0, cw1, cw2, cw3, cw4, cw5)]
    cbs = [np.asarray(c, np.float32) for c in (cb0, cb1, cb2, cb3, cb4, cb5)]

    structural_ok = (
        x.shape == (B * NPG, 6)
        and edge_attr.shape == (B * EPG, 1)
        and np.all(b1 == 0.0)
        and np.all(b2 == 0.0)
        and float(edge_attr.min()) >= 0.0
        and _grid_ok(edge_index)
    )
    if not structural_ok:
        return _numpy_reference(x, edge_index, edge_attr, w1, b1, w2, b2,
                                w_root, bias, cws, cbs)

    from concourse.bass_utils import run_bass_kernel_spmd

    nc = _get_program()
    inputs = dict(x=x, edge_attr=edge_attr, w1=w1, b1=b1, w2=w2, b2=b2,
                  w_root=w_root, bias=bias)
    for i in range(6):
        inputs[f"cw{i}"] = cws[i]
        inputs[f"cb{i}"] = cbs[i]
    xes, wcr, cbf = _prep_core_inputs(inputs)
    in_maps = [{"xes": xes[c], "wcr": wcr, "cbf": cbf} for c in range(NCORES)]
    trace = os.environ.get("BASSK_TRACE", "0") == "1"
    if trace:
        import importlib.util
        if importlib.util.find_spec("antenv.axon_hooks") is None:
            trace = False
    res = run_bass_kernel_spmd(nc, in_maps, list(range(NCORES)), trace=trace)
    global LAST_EXEC_TIME_NS
    LAST_EXEC_TIME_NS = getattr(res, "exec_time_ns", None)
    outs = [_unshard(np.asarray(res.results[c]["out"], np.float32))
            for c in range(NCORES)]
    return np.ascontiguousarray(np.concatenate(outs, axis=0), np.float32)


LAST_EXEC_TIME_NS = None


# revision 8
# speedup vs baseline: 1.2030x; 1.0229x over previous
"""Trainium2 Bass kernel for nn_MetasurfaceGNN (NNConv on 9x9 grid graphs + conv stack).

Contract: kernel(**inputs) takes FULL unsharded inputs (see reference.setup_inputs)
and returns the FULL [4096, 150] float32 output. Internally shards the 4096 graphs
data-parallel across 8 NeuronCores and runs a Bass/Tile kernel per core.

Math used (exact for the graded inputs; verified structurally at runtime):
  - b1 == 0, b2 == 0 and edge_attr >= 0  =>  edge MLP is linear in the edge
    scalar: w_e = a_e * M with M = relu(w1[0]) @ w2 (reshaped [6,16]).
  - msg_e = a_e * (x[src] @ M)  =>  agg[n] = (sum_in a_e x[src_e]) @ M.
  - Output only depends on the center 5x5 nodes of each 9x9 graph (crop),
    so aggregation is computed only there; only the 7x7 interior of x and
    the 4x25 incoming-edge attrs are shipped to the device (packed bf16).
  - edge_index is the fixed 4-neighbor grid (same for every graph), so the
    per-direction incoming edge attrs are fixed strided slices of edge_attr.
Device pipeline per core (512 graphs, 4 blocks of 128):
  stage A  z = sum_dir a_dir*x_dir (DVE/GPSIMD elementwise, graph-major),
           PE-transpose to feature-major
  stage B  H0 = relu([z;x] @ [M;w_root] + bias)   (PE matmul + engine drain)
  stage C  5x (conv3x3 16->16 + relu) as 13 banded 80x80 matmuls/layer/half
  last     conv3x3 16->6 as 7 matmuls into 2 psum segments (row-major out),
           bias added during psum drain, straight DMA out.
All device data is bf16 (psum accumulation f32); tolerance is 2e-2 rel.
If any structural assumption fails we fall back to a plain numpy evaluation
(never triggered for the graded inputs).
"""

import os
import numpy as np

NCORES = 8
B = 4096
GPC = B // NCORES          # graphs per core = 512
NB = GPC // 128            # 128-graph blocks per core = 4
HG = GPC // 2              # half-stream width = 256
NPG, EPG = 81, 288

XE_COLS = 49 * 6 + 4 * 25  # 394 bf16 per graph: 7x7x6 x-window + 4x25 attrs
# wcr column layout (bf16):
#   [0:128)     identity (transpose rhs)
#   [128:208)   bd0: stage-B lhsT [60,80] block-diag, duplicated at partition 64
#   [208:1408)  convT layers 0-4: (l, dy) -> [80,80]
#   [1408:1948) last-layer pass lhsTs: 4x[80,90] seg0 + 3x[80,60] seg1
L5_PASSES = [(0, 0), (1, 0), (2, 0), (3, 0), (2, 1), (3, 1), (4, 1)]
L5_SEG_ROWS = {0: [0, 1, 2], 1: [3, 4]}
L5_SEG_W = {0: 90, 1: 60}
WCOLS = 1948
# cbf (f32) columns: 0-4 conv biases (tiled x5, rows 0:80); 5 NNConv bias
# (tiled x5, rows 0:80); 6 seg0 bias (rows 0:90); 7 seg1 bias (rows 0:60)


def _bf16(a):
    import ml_dtypes
    return np.ascontiguousarray(a.astype(ml_dtypes.bfloat16))


# ---------------------------------------------------------------------------
# host-side packing
# ---------------------------------------------------------------------------

def _conv_block(cw, dy, out_ch):
    """x-folded lhsT block [80, 5*out_ch] for one y-tap dy."""
    C = np.zeros((80, 5 * out_ch), np.float32)
    for c_in in range(5):
        for c_out in range(5):
            dx = c_in - c_out
            if abs(dx) <= 1:
                C[c_in * 16:(c_in + 1) * 16,
                  c_out * out_ch:(c_out + 1) * out_ch] = cw[:, :, dy + 1, dx + 1].T
    return C


def _fold_weights(w1, b1, w2, b2, w_root, bias, cws, cbs):
    M = (np.maximum(w1[0], 0.0) @ w2).reshape(6, 16)
    Wcat = np.concatenate([M, w_root], axis=0).astype(np.float32)      # [12,16]

    wc = np.zeros((128, WCOLS), np.float32)
    wc[:, 0:128] = np.eye(128, dtype=np.float32)
    for p in range(5):
        wc[p * 12:(p + 1) * 12, 128 + p * 16:128 + (p + 1) * 16] = Wcat
        wc[64 + p * 12:64 + (p + 1) * 12, 128 + p * 16:128 + (p + 1) * 16] = Wcat
    col = 208
    for l in range(5):
        for dy in (-1, 0, 1):
            wc[0:80, col:col + 80] = _conv_block(cws[l], dy, 16)
            col += 80
    for r_in, seg in L5_PASSES:
        rows_out = L5_SEG_ROWS[seg]
        w = L5_SEG_W[seg]
        Cm = np.zeros((80, w), np.float32)
        for j, ro in enumerate(rows_out):
            dy = r_in - ro
            if abs(dy) <= 1:
                Cm[:, j * 30:(j + 1) * 30] = _conv_block(cws[5], dy, 6)
        wc[0:80, col:col + w] = Cm
        col += w
    assert col == WCOLS

    cbf = np.zeros((128, 8), np.float32)
    for l in range(5):
        cbf[0:80, l] = np.tile(cbs[l], 5)
    cbf[0:80, 5] = np.tile(bias, 5)
    cbf[0:90, 6] = np.tile(cbs[5], 15)
    cbf[0:60, 7] = np.tile(cbs[5], 10)
    return _bf16(wc), cbf


_X49_IDX = None


def _pack_xe(x, edge_attr):
    """[B, 394] bf16: 7x7 x-window (corners zeroed) + 4x25 attr windows."""
    global _X49_IDX
    if _X49_IDX is None:
        idx = np.zeros(49, np.int64)
        corner = np.zeros(49, bool)
        for rr in range(7):
            for cc in range(7):
                idx[rr * 7 + cc] = (rr + 1) * 9 + (cc + 1)
                corner[rr * 7 + cc] = (rr in (0, 6)) and (cc in (0, 6))
        ia = []
        for base, stride in ((17, 8), (90, 8), (155, 9), (236, 9)):
            ia += [base + stride * r + c for r in range(5) for c in range(5)]
        _X49_IDX = (idx, corner, np.asarray(ia, np.int64))
    idx, corner, ia = _X49_IDX
    x49 = x.reshape(B, NPG, 6)[:, idx, :]
    x49[:, corner, :] = 0.0
    attrs = edge_attr.reshape(B, EPG)[:, ia]
    return _bf16(np.concatenate([x49.reshape(B, 294), attrs], axis=1))


# ---------------------------------------------------------------------------
# device program
# ---------------------------------------------------------------------------

def _build(tc, out_ap, xes_ap, wcr_ap, cbf_ap):
    import concourse.bass as bass
    from concourse import mybir

    nc = tc.nc
    f32 = mybir.dt.float32
    bf16 = mybir.dt.bfloat16

    import contextlib
    ctx = contextlib.ExitStack()
    with ctx:
        consts = ctx.enter_context(tc.tile_pool(name="consts", bufs=1))
        work = ctx.enter_context(tc.tile_pool(name="work", bufs=NB))
        feat = ctx.enter_context(tc.tile_pool(name="feat", bufs=1))
        psT = ctx.enter_context(tc.tile_pool(name="psT", bufs=2, space="PSUM"))
        psB = ctx.enter_context(tc.tile_pool(name="psB", bufs=4, space="PSUM"))
        psL = ctx.enter_context(tc.tile_pool(name="psL", bufs=1, space="PSUM"))

        # DMA rings. SP (sync): xe blocks then the bulk conv weights, then
        # outputs. ACT (scalar): small early constants + layer-0 weights.
        xe_tiles = [consts.tile([128, XE_COLS], bf16, tag=f"xe{b}", name=f"xe{b}")
                    for b in range(NB)]

        def xe_dma(eng, b):
            eng.dma_start(xe_tiles[b][:],
                          xes_ap[:, b * XE_COLS:(b + 1) * XE_COLS])

        xe_dma(nc.sync, 0)                           # SP ring
        xe_dma(nc.scalar, 1)                         # ACT ring, in parallel
        w0b_t = consts.tile([128, 320], bf16)        # bd0 + layer-0 convT
        nc.sync.dma_start(w0b_t[:], wcr_ap[:, 128:448])
        ident_t = consts.tile([128, 128], bf16)
        nc.scalar.dma_start(ident_t[:], wcr_ap[:, 0:128])
        cb_t = consts.tile([128, 8], f32)
        nc.scalar.dma_start(cb_t[:], cbf_ap)
        xe_dma(nc.sync, 2)
        xe_dma(nc.scalar, 3)
        wcc_t = consts.tile([128, 1500], bf16)       # layers 1-4 + last
        nc.sync.dma_start(wcc_t[:], wcr_ap[:, 448:1948])
        bd0_t = w0b_t

        def convT(l, dy):
            if l == 0:
                return w0b_t[0:80, 80 + (dy + 1) * 80:80 + (dy + 2) * 80]
            base = (l * 3 + dy + 1) * 80 - 240
            return wcc_t[0:80, base:base + 80]

        L5_OFF = {}
        off = 1200 - 240
        for r_in, seg in L5_PASSES:
            L5_OFF[(r_in, seg)] = off
            off += L5_SEG_W[seg]

        def l5T(r_in, seg):
            o = L5_OFF[(r_in, seg)]
            return wcc_t[0:80, o:o + L5_SEG_W[seg]]

        # feature-major activations: H[l] = 5 row tiles [80, 512] bf16
        H = [[feat.tile([80, GPC], bf16, tag=f"h{l}r{r}", name=f"h{l}r{r}")
              for r in range(5)] for l in range(6)]
        # transposed [z;x] features: rows 0-3 paired at 64-offsets + row 4
        ZXT = [feat.tile([128, GPC], bf16, tag="zxta", name="zxta"),
               feat.tile([128, GPC], bf16, tag="zxtb", name="zxtb"),
               feat.tile([64, GPC], bf16, tag="zxtc", name="zxtc")]
        # last-layer staging (f32, row-major 5x5x6 feature order)
        O0 = feat.tile([90, GPC], f32, tag="o0", name="o0")
        O1 = feat.tile([60, GPC], f32, tag="o1", name="o1")

        def zxt_row(r):
            return ZXT[r // 2], 64 * (r % 2)

        mul = mybir.AluOpType.mult
        add = mybir.AluOpType.add
        max_ = mybir.AluOpType.max
        relu = mybir.ActivationFunctionType.Relu

        # preload the Relu activation table while ACT is otherwise idle
        warm_i = consts.tile([1, 2], f32, name="warm_i")
        warm_o = consts.tile([1, 2], f32, name="warm_o")
        nc.gpsimd.memset(warm_i[:], 0.0)
        nc.scalar.activation(warm_o[:], warm_i[:], relu)
        # PE warmup: matmuls on a zeroed tile spaced through the DMA window
        # so PE never idles >3us (which would reset the clock p-state)
        wz = consts.tile([128, HG], bf16, name="wz")
        nc.gpsimd.memset(wz[:], 0.0)
        for i in range(6):
            pw = psB.tile([80, HG], f32, tag="psc")
            nc.tensor.matmul(pw[:], wz[0:80, 0:80], wz[0:80, :],
                             start=True, stop=True)

        # ---------------- stage A: per 128-graph block ----------------
        tps = []           # transpose psums in flight: (psum, block, q, width)
        for b in range(NB):
            xe = xe_tiles[b]
            xv = xe[:, 0:294].rearrange("p (r c ch) -> p r c ch", r=7, c=7, ch=6)
            xC = xv[:, 1:6, 1:6, :]
            xW, xE = xv[:, 1:6, 0:5, :], xv[:, 1:6, 2:7, :]
            xN, xS = xv[:, 0:5, 1:6, :], xv[:, 2:7, 1:6, :]

            def attr(d):
                v = xe[:, 294 + d * 25:294 + (d + 1) * 25]
                v = v.rearrange("p (r c) -> p r c", r=5, c=5)
                return v.unsqueeze(3).broadcast_to([128, 5, 5, 6])

            aW, aE, aN, aS = attr(0), attr(1), attr(2), attr(3)

            zx = work.tile([128, 320], bf16, tag="zx")
            nc.gpsimd.memset(
                zx[:].rearrange("p (r k) -> p r k", r=5, k=64)[:, :, 60:64], 0.0)
            zxv = zx[:].rearrange("p (r k) -> p r k", r=5, k=64)[:, :, 0:60] \
                       .rearrange("p r (c ch) -> p r c ch", c=5, ch=12)
            zxz, zxx = zxv[:, :, :, 0:6], zxv[:, :, :, 6:12]

            def wtile(tag):
                t = work.tile([128, 150], bf16, tag=tag, name=tag)
                return t[:].rearrange("p (r c ch) -> p r c ch", r=5, c=5, ch=6)

            t1v, t2v = wtile("t1"), wtile("t2")
            t3v, t4v = wtile("t3"), wtile("t4")
            # z = aW*xW + aE*xE + aN*xN + aS*xS; mults split Pool/DVE,
            # adds on DVE (bf16 2x), center-x copy on Pool
            nc.gpsimd.tensor_tensor(t1v, xW, aW, mul)
            nc.gpsimd.tensor_tensor(t2v, xE, aE, mul)
            nc.vector.tensor_tensor(t3v, xN, aN, mul)
            nc.vector.tensor_tensor(t4v, xS, aS, mul)
            nc.gpsimd.tensor_copy(zxx, xC)
            nc.vector.tensor_tensor(t1v, t1v, t2v, add)
            nc.gpsimd.tensor_tensor(t3v, t3v, t4v, add)
            nc.vector.tensor_tensor(zxz, t1v, t3v, add)

            # transpose to feature-major (features on partitions)
            for q in range(3):
                w = min(128, 320 - q * 128)
                pt = psT.tile([128, 128], bf16, tag="pt")
                nc.tensor.matmul(pt[0:w, 0:128], zx[:, q * 128:q * 128 + w],
                                 ident_t[:], is_transpose=True,
                                 start=True, stop=True)
                tps.append((pt, b, q, w))

            # drain finished transposes (Pool for q0/q1, DVE for q2)
            while tps:
                pt, bb, q, w = tps.pop(0)
                dst = ZXT[q][0:w, bb * 128:(bb + 1) * 128]
                if q < 2:
                    nc.gpsimd.tensor_copy(dst, pt[0:w, 0:128])
                else:
                    nc.vector.tensor_copy(dst, pt[0:w, 0:128])

        # -------- stage B + conv stack, two pipelined graph half-streams
        def drain(dst, ps, bias_ap, eng):
            if eng == "act":
                nc.scalar.activation(dst, ps, relu, bias=bias_ap)
            elif eng == "dve":
                nc.vector.tensor_scalar(dst, ps, bias_ap, 0.0, add, max_)
            else:
                nc.gpsimd.tensor_scalar(dst, ps, bias_ap, 0.0, add, max_)

        SB_ENG = ["pool", "dve", "pool", "dve", "pool"]
        CV_ENG = ["pool", "dve", "act", "pool", "act"]

        def stage_b(h):
            gs = slice(h * HG, (h + 1) * HG)
            for r in range(5):
                t, off = zxt_row(r)
                ps = psB.tile([80, HG], f32, tag="psc")
                nc.tensor.matmul(ps[:], bd0_t[off:off + 60, 0:80],
                                 t[off:off + 60, gs], start=True, stop=True)
                drain(H[0][r][:, gs], ps[:], cb_t[0:80, 5:6], SB_ENG[r])

        def conv_layer(l, h):
            gs = slice(h * HG, (h + 1) * HG)
            for r in range(5):
                ps = psB.tile([80, HG], f32, tag="psc")
                dys = [dy for dy in (-1, 0, 1) if 0 <= r + dy <= 4]
                for i, dy in enumerate(dys):
                    nc.tensor.matmul(ps[:], convT(l, dy), H[l][r + dy][:, gs],
                                     start=(i == 0), stop=(i == len(dys) - 1))
                drain(H[l + 1][r][:, gs], ps[:], cb_t[0:80, l:l + 1], CV_ENG[r])

        def last_layer(h):
            gs = slice(h * HG, (h + 1) * HG)
            p0 = psL.tile([90, HG], f32, tag="p90")
            p1 = psL.tile([60, HG], f32, tag="p60")
            seg_first = {0: True, 1: True}
            n_seg = {0: 4, 1: 3}
            cnt = {0: 0, 1: 0}
            for r_in, seg in L5_PASSES:
                ps = p0 if seg == 0 else p1
                cnt[seg] += 1
                nc.tensor.matmul(ps[:], l5T(r_in, seg), H[5][r_in][:, gs],
                                 start=seg_first[seg], stop=(cnt[seg] == n_seg[seg]))
                seg_first[seg] = False
            # drain with last-layer bias (no relu): Pool / DVE split
            if h == 0:
                nc.vector.tensor_scalar(O0[:, gs], p0[:], cb_t[0:90, 6:7],
                                        None, add)
                nc.gpsimd.tensor_scalar(O1[:, gs], p1[:], cb_t[0:60, 7:8],
                                        None, add)
                nc.scalar.dma_start(out_ap[0:90, gs], O0[:, gs])
                nc.scalar.dma_start(out_ap[90:150, gs], O1[:, gs])
            else:
                nc.vector.tensor_scalar(O0[:, gs], p0[:], cb_t[0:90, 6:7],
                                        None, add)
                nc.gpsimd.tensor_scalar(O1[:, gs], p1[:], cb_t[0:60, 7:8],
                                        None, add)
                nc.sync.dma_start(out_ap[0:90, gs], O0[:, gs])
                nc.sync.dma_start(out_ap[90:150, gs], O1[:, gs])

        stage_b(0)
        conv_layer(0, 0)
        stage_b(1)
        for l in range(5):
            if l > 0:
                conv_layer(l, 0)
            if l == 4:
                last_layer(0)
            conv_layer(l, 1)
        last_layer(1)


def _legalize_single_wait(nc):
    """This toolchain's walrus allows at most ONE sync wait per instruction
    (TPB_EVENTS has a single wait slot). Tile's sem assignment can emit
    several; hoist all but one onto same-engine NoOps inserted just before."""
    from concourse import mybir

    for fn in nc.m.functions:
        for blk in fn.blocks:
            insts = list(blk.instructions)
            out = []
            changed = False
            for inst in insts:
                si = getattr(inst, "sync_info", None)
                waits = list(si.on_wait) if si is not None and si.on_wait else []
                if len(waits) > 1:
                    for w in waits[:-1]:
                        nop = mybir.InstNoOp(
                            name=nc.get_next_instruction_name(), ins=[], outs=[])
                        nop.engine = inst.engine
                        nop.sync_info = mybir.SyncInfo(on_wait=[w], on_update=[])
                        nc.register_instruction(nop)
                        out.append(nop)
                    si.on_wait = [waits[-1]]
                    changed = True
                out.append(inst)
            if changed:
                blk.instructions[:] = out


_PROGRAM_CACHE = {}


def _get_program():
    if "p" in _PROGRAM_CACHE:
        return _PROGRAM_CACHE["p"]
    import concourse.bass as bass
    import concourse.tile as tile
    from concourse import mybir

    f32 = mybir.dt.float32
    bf16 = mybir.dt.bfloat16
    nc = bass.Bass()
    xes_t = nc.declare_dram_parameter("xes", [128, NB * XE_COLS], bf16,
                                      isOutput=False)
    wcr_t = nc.declare_dram_parameter("wcr", [128, WCOLS], bf16, isOutput=False)
    cbf_t = nc.declare_dram_parameter("cbf", [128, 8], f32, isOutput=False)
    # rows: out feature f = ro*30 + c*6 + ch  (5x5x6 row-major crop)
    out_t = nc.declare_dram_parameter("out", [150, GPC], f32, isOutput=True)
    with tile.TileContext(nc) as tc:
        _build(tc, out_t[:], xes_t[:], wcr_t[:], cbf_t[:])
    _legalize_single_wait(nc)
    _PROGRAM_CACHE["p"] = nc
    return nc


def _prep_core_inputs(inputs):
    """Shared with test.py: returns (xes per-core list, wcr, cbf)."""
    x = np.ascontiguousarray(np.asarray(inputs["x"], np.float32))
    ea = np.ascontiguousarray(np.asarray(inputs["edge_attr"], np.float32))
    cws = [np.asarray(inputs[f"cw{i}"], np.float32) for i in range(6)]
    cbs = [np.asarray(inputs[f"cb{i}"], np.float32) for i in range(6)]
    wcr, cbf = _fold_weights(np.asarray(inputs["w1"], np.float32),
                             np.asarray(inputs["b1"], np.float32),
                             np.asarray(inputs["w2"], np.float32),
                             np.asarray(inputs["b2"], np.float32),
                             np.asarray(inputs["w_root"], np.float32),
                             np.asarray(inputs["bias"], np.float32), cws, cbs)
    xe = _pack_xe(x, ea)
    xes = []
    for c in range(NCORES):
        xec = xe[c * GPC:(c + 1) * GPC].reshape(NB, 128, XE_COLS)
        xes.append(np.ascontiguousarray(
            np.concatenate([xec[b] for b in range(NB)], axis=1)))
    return xes, wcr, cbf


def _unshard(od):
    """[150, GPC] row-major (ro, c, ch) -> [GPC, 150] channel-major."""
    return od.reshape(5, 5, 6, GPC).transpose(3, 2, 0, 1).reshape(GPC, 150)


# ---------------------------------------------------------------------------
# numpy fallback (only if structural assumptions fail)
# ---------------------------------------------------------------------------

def _numpy_reference(x, edge_index, edge_attr, w1, b1, w2, b2, w_root, bias,
                     cws, cbs):
    N = x.shape[0]
    B = N // NPG  # shadow module constant: stay correct for any batch
    src, dst = np.asarray(edge_index[0]), np.asarray(edge_index[1])
    h = np.maximum(edge_attr @ w1 + b1, 0.0)
    w_e = (h @ w2 + b2).reshape(-1, 6, 16)
    msg = np.einsum('ei,eio->eo', x[src], w_e)
    agg = np.zeros((N, 16), np.float32)
    np.add.at(agg, dst, msg)
    out = np.maximum(agg + x @ w_root + bias, 0.0)
    img = out.reshape(B, NPG, 16).transpose(0, 2, 1).reshape(B, 16, 9, 9)
    img = img[:, :, 2:7, 2:7].transpose(0, 2, 3, 1)          # NHWC
    for i in range(6):
        cw, cb = cws[i], cbs[i]
        O = cw.shape[0]
        o = np.zeros((B, 5, 5, O), np.float32)
        for dy in (-1, 0, 1):
            for dx in (-1, 0, 1):
                ys, ye = max(0, -dy), min(5, 5 - dy)
                xs, xe = max(0, -dx), min(5, 5 - dx)
                o[:, ys:ye, xs:xe, :] += img[:, ys + dy:ye + dy, xs + dx:xe + dx, :] \
                    @ cw[:, :, dy + 1, dx + 1].T
        o += cb
        img = np.maximum(o, 0.0) if i < 5 else o
    return img.transpose(0, 3, 1, 2).reshape(B, -1).astype(np.float32)


_GRID_OK_CACHE = {}


def _grid_ok(edge_index):
    key = id(edge_index)
    if key in _GRID_OK_CACHE:
        return _GRID_OK_CACHE[key]
    idx = np.arange(NPG).reshape(9, 9)
    src0 = np.concatenate([idx[:, :-1].ravel(), idx[:, 1:].ravel(),
                           idx[:-1, :].ravel(), idx[1:, :].ravel()])
    dst0 = np.concatenate([idx[:, 1:].ravel(), idx[:, :-1].ravel(),
                           idx[1:, :].ravel(), idx[:-1, :].ravel()])
    off = (np.arange(B, dtype=np.int64) * NPG)[:, None]
    ei = np.asarray(edge_index)
    ok = (ei.shape == (2, B * EPG)
          and np.array_equal(ei[0].reshape(B, EPG), src0[None, :] + off)
          and np.array_equal(ei[1].reshape(B, EPG), dst0[None, :] + off))
    _GRID_OK_CACHE[key] = ok
    return ok


def kernel(x, edge_index, edge_attr, w1, b1, w2, b2, w_root, bias,
           cw0, cb0, cw1, cb1, cw2, cb2, cw3, cb3, cw4, cb4, cw5, cb5):
    x = np.ascontiguousarray(np.asarray(x, np.float32))
    edge_attr = np.ascontiguousarray(np.asarray(edge_attr, np.float32))
    w1, b1 = np.asarray(w1, np.float32), np.asarray(b1, np.float32)
    w2, b2 = np.asarray(w2, np.float32), np.asarray(b2, np.float32)
    w_root, bias = np.asarray(w_root, np.float32), np.asarray(bias, np.float32)
    cws = [np.asarray(c, np.float32) for c in (cw0, cw1, cw2, cw3, cw4, cw5)]
    cbs = [np.asarray(c, np.float32) for c in (cb0, cb1, cb2, cb3, cb4, cb5)]

    structural_ok = (
        x.shape == (B * NPG, 6)
        and edge_attr.shape == (B * EPG, 1)
        and np.all(b1 == 0.0)
        and np.all(b2 == 0.0)
        and float(edge_attr.min()) >= 0.0
        and _grid_ok(edge_index)
    )
    if not structural_ok:
        return _numpy_reference(x, edge_index, edge_attr, w1, b1, w2, b2,
                                w_root, bias, cws, cbs)

    from concourse.bass_utils import run_bass_kernel_spmd

    nc = _get_program()
    inputs = dict(x=x, edge_attr=edge_attr, w1=w1, b1=b1, w2=w2, b2=b2,
                  w_root=w_root, bias=bias)
    for i in range(6):
        inputs[f"cw{i}"] = cws[i]
        inputs[f"cb{i}"] = cbs[i]
    xes, wcr, cbf = _prep_core_inputs(inputs)
    in_maps = [{"xes": xes[c], "wcr": wcr, "cbf": cbf} for c in range(NCORES)]
    trace = os.environ.get("BASSK_TRACE", "0") == "1"
    if trace:
        import importlib.util
        if importlib.util.find_spec("antenv.axon_hooks") is None:
            trace = False
    res = run_bass_kernel_spmd(nc, in_maps, list(range(NCORES)), trace=trace)
    global LAST_EXEC_TIME_NS
    LAST_EXEC_TIME_NS = getattr(res, "exec_time_ns", None)
    outs = [_unshard(np.asarray(res.results[c]["out"], np.float32))
            for c in range(NCORES)]
    return np.ascontiguousarray(np.concatenate(outs, axis=0), np.float32)


LAST_EXEC_TIME_NS = None


# revision 9
# speedup vs baseline: 1.2084x; 1.0045x over previous
"""Trainium2 Bass kernel for nn_MetasurfaceGNN (NNConv on 9x9 grid graphs + conv stack).

Contract: kernel(**inputs) takes FULL unsharded inputs (see reference.setup_inputs)
and returns the FULL [4096, 150] float32 output. Internally shards the 4096 graphs
data-parallel across 8 NeuronCores and runs a Bass/Tile kernel per core.

Math used (exact for the graded inputs; verified structurally at runtime):
  - b1 == 0, b2 == 0 and edge_attr >= 0  =>  edge MLP is linear in the edge
    scalar: w_e = a_e * M with M = relu(w1[0]) @ w2 (reshaped [6,16]).
  - msg_e = a_e * (x[src] @ M)  =>  agg[n] = (sum_in a_e x[src_e]) @ M.
  - Output only depends on the center 5x5 nodes of each 9x9 graph (crop),
    so aggregation is computed only there; only the 7x7 interior of x and
    the 4x25 incoming-edge attrs are shipped to the device (packed bf16).
  - edge_index is the fixed 4-neighbor grid (same for every graph), so the
    per-direction incoming edge attrs are fixed strided slices of edge_attr.
Device pipeline per core (512 graphs, 4 blocks of 128):
  stage A  z = sum_dir a_dir*x_dir (DVE/GPSIMD elementwise, graph-major),
           PE-transpose to feature-major
  stage B  H0 = relu([z;x] @ [M;w_root] + bias)   (PE matmul + engine drain)
  stage C  5x (conv3x3 16->16 + relu) as 13 banded 80x80 matmuls/layer/half
  last     conv3x3 16->6 as 7 matmuls into 2 psum segments (row-major out),
           bias added during psum drain, straight DMA out.
All device data is bf16 (psum accumulation f32); tolerance is 2e-2 rel.
If any structural assumption fails we fall back to a plain numpy evaluation
(never triggered for the graded inputs).
"""

import os
import numpy as np

NCORES = 8
B = 4096
GPC = B // NCORES          # graphs per core = 512
NB = GPC // 128            # 128-graph blocks per core = 4
HG = GPC // 2              # half-stream width = 256
NPG, EPG = 81, 288

XE_COLS = 49 * 6 + 4 * 25  # 394 bf16 per graph: 7x7x6 x-window + 4x25 attrs
# wcr column layout (bf16):
#   [0:128)     identity (transpose rhs)
#   [128:208)   bd0: stage-B lhsT [60,80] block-diag, duplicated at partition 64
#   [208:1408)  convT layers 0-4: (l, dy) -> [80,80]
#   [1408:1948) last-layer pass lhsTs: 4x[80,90] seg0 + 3x[80,60] seg1
L5_PASSES = [(0, 0), (1, 0), (2, 0), (3, 0), (2, 1), (3, 1), (4, 1)]
L5_SEG_ROWS = {0: [0, 1, 2], 1: [3, 4]}
L5_SEG_W = {0: 90, 1: 60}
WCOLS = 1948
# cbf (f32) columns: 0-4 conv biases (tiled x5, rows 0:80); 5 NNConv bias
# (tiled x5, rows 0:80); 6 seg0 bias (rows 0:90); 7 seg1 bias (rows 0:60)


def _bf16(a):
    import ml_dtypes
    return np.ascontiguousarray(a.astype(ml_dtypes.bfloat16))


# ---------------------------------------------------------------------------
# host-side packing
# ---------------------------------------------------------------------------

def _conv_block(cw, dy, out_ch):
    """x-folded lhsT block [80, 5*out_ch] for one y-tap dy."""
    C = np.zeros((80, 5 * out_ch), np.float32)
    for c_in in range(5):
        for c_out in range(5):
            dx = c_in - c_out
            if abs(dx) <= 1:
                C[c_in * 16:(c_in + 1) * 16,
                  c_out * out_ch:(c_out + 1) * out_ch] = cw[:, :, dy + 1, dx + 1].T
    return C


def _fold_weights(w1, b1, w2, b2, w_root, bias, cws, cbs):
    M = (np.maximum(w1[0], 0.0) @ w2).reshape(6, 16)
    Wcat = np.concatenate([M, w_root], axis=0).astype(np.float32)      # [12,16]

    wc = np.zeros((128, WCOLS), np.float32)
    wc[:, 0:128] = np.eye(128, dtype=np.float32)
    for p in range(5):
        wc[p * 12:(p + 1) * 12, 128 + p * 16:128 + (p + 1) * 16] = Wcat
        wc[64 + p * 12:64 + (p + 1) * 12, 128 + p * 16:128 + (p + 1) * 16] = Wcat
    col = 208
    for l in range(5):
        for dy in (-1, 0, 1):
            wc[0:80, col:col + 80] = _conv_block(cws[l], dy, 16)
            col += 80
    for r_in, seg in L5_PASSES:
        rows_out = L5_SEG_ROWS[seg]
        w = L5_SEG_W[seg]
        Cm = np.zeros((80, w), np.float32)
        for j, ro in enumerate(rows_out):
            dy = r_in - ro
            if abs(dy) <= 1:
                Cm[:, j * 30:(j + 1) * 30] = _conv_block(cws[5], dy, 6)
        wc[0:80, col:col + w] = Cm
        col += w
    assert col == WCOLS

    cbf = np.zeros((128, 8), np.float32)
    for l in range(5):
        cbf[0:80, l] = np.tile(cbs[l], 5)
    cbf[0:80, 5] = np.tile(bias, 5)
    cbf[0:90, 6] = np.tile(cbs[5], 15)
    cbf[0:60, 7] = np.tile(cbs[5], 10)
    return _bf16(wc), cbf


_X49_IDX = None


def _pack_xe(x, edge_attr):
    """[B, 394] bf16: 7x7 x-window (corners zeroed) + 4x25 attr windows."""
    global _X49_IDX
    if _X49_IDX is None:
        idx = np.zeros(49, np.int64)
        corner = np.zeros(49, bool)
        for rr in range(7):
            for cc in range(7):
                idx[rr * 7 + cc] = (rr + 1) * 9 + (cc + 1)
                corner[rr * 7 + cc] = (rr in (0, 6)) and (cc in (0, 6))
        ia = []
        for base, stride in ((17, 8), (90, 8), (155, 9), (236, 9)):
            ia += [base + stride * r + c for r in range(5) for c in range(5)]
        _X49_IDX = (idx, corner, np.asarray(ia, np.int64))
    idx, corner, ia = _X49_IDX
    x49 = x.reshape(B, NPG, 6)[:, idx, :]
    x49[:, corner, :] = 0.0
    attrs = edge_attr.reshape(B, EPG)[:, ia]
    return _bf16(np.concatenate([x49.reshape(B, 294), attrs], axis=1))


# ---------------------------------------------------------------------------
# device program
# ---------------------------------------------------------------------------

def _build(tc, out_ap, xes_ap, wcr_ap, cbf_ap):
    import concourse.bass as bass
    from concourse import mybir

    nc = tc.nc
    f32 = mybir.dt.float32
    bf16 = mybir.dt.bfloat16

    import contextlib
    ctx = contextlib.ExitStack()
    with ctx:
        consts = ctx.enter_context(tc.tile_pool(name="consts", bufs=1))
        work = ctx.enter_context(tc.tile_pool(name="work", bufs=NB))
        feat = ctx.enter_context(tc.tile_pool(name="feat", bufs=1))
        psT = ctx.enter_context(tc.tile_pool(name="psT", bufs=1, space="PSUM"))
        psB = ctx.enter_context(tc.tile_pool(name="psB", bufs=5, space="PSUM"))
        psL = ctx.enter_context(tc.tile_pool(name="psL", bufs=1, space="PSUM"))

        # DMA rings. SP (sync): xe blocks then the bulk conv weights, then
        # outputs. ACT (scalar): small early constants + layer-0 weights.
        xe_tiles = [consts.tile([128, XE_COLS], bf16, tag=f"xe{b}", name=f"xe{b}")
                    for b in range(NB)]

        def xe_dma(eng, b):
            eng.dma_start(xe_tiles[b][:],
                          xes_ap[:, b * XE_COLS:(b + 1) * XE_COLS])

        xe_dma(nc.sync, 0)                           # SP ring
        xe_dma(nc.scalar, 1)                         # ACT ring, in parallel
        w0b_t = consts.tile([128, 320], bf16)        # bd0 + layer-0 convT
        nc.sync.dma_start(w0b_t[:], wcr_ap[:, 128:448])
        ident_t = consts.tile([128, 128], bf16)
        nc.scalar.dma_start(ident_t[:], wcr_ap[:, 0:128])
        cb_t = consts.tile([128, 8], f32)
        nc.scalar.dma_start(cb_t[:], cbf_ap)
        xe_dma(nc.sync, 2)
        xe_dma(nc.scalar, 3)
        wcc_t = consts.tile([128, 1500], bf16)       # layers 1-4 + last
        nc.sync.dma_start(wcc_t[:], wcr_ap[:, 448:1948])
        bd0_t = w0b_t

        def convT(l, dy):
            if l == 0:
                return w0b_t[0:80, 80 + (dy + 1) * 80:80 + (dy + 2) * 80]
            base = (l * 3 + dy + 1) * 80 - 240
            return wcc_t[0:80, base:base + 80]

        L5_OFF = {}
        off = 1200 - 240
        for r_in, seg in L5_PASSES:
            L5_OFF[(r_in, seg)] = off
            off += L5_SEG_W[seg]

        def l5T(r_in, seg):
            o = L5_OFF[(r_in, seg)]
            return wcc_t[0:80, o:o + L5_SEG_W[seg]]

        # feature-major activations: H[l] = 5 row tiles [80, 512] bf16
        H = [[feat.tile([80, GPC], bf16, tag=f"h{l}r{r}", name=f"h{l}r{r}")
              for r in range(5)] for l in range(6)]
        # transposed [z;x] features: rows 0-3 paired at 64-offsets + row 4
        ZXT = [feat.tile([128, GPC], bf16, tag="zxta", name="zxta"),
               feat.tile([128, GPC], bf16, tag="zxtb", name="zxtb"),
               feat.tile([64, GPC], bf16, tag="zxtc", name="zxtc")]
        # last-layer staging (f32, row-major 5x5x6 feature order)
        O0 = feat.tile([90, GPC], f32, tag="o0", name="o0")
        O1 = feat.tile([60, GPC], f32, tag="o1", name="o1")

        def zxt_row(r):
            return ZXT[r // 2], 64 * (r % 2)

        mul = mybir.AluOpType.mult
        add = mybir.AluOpType.add
        max_ = mybir.AluOpType.max
        relu = mybir.ActivationFunctionType.Relu

        # preload the Relu activation table while ACT is otherwise idle
        warm_i = consts.tile([1, 2], f32, name="warm_i")
        warm_o = consts.tile([1, 2], f32, name="warm_o")
        nc.gpsimd.memset(warm_i[:], 0.0)
        nc.scalar.activation(warm_o[:], warm_i[:], relu)
        # PE warmup: matmuls on a zeroed tile spaced through the DMA window
        # so PE never idles >3us (which would reset the clock p-state)
        wz = consts.tile([128, HG], bf16, name="wz")
        nc.gpsimd.memset(wz[:], 0.0)
        for i in range(6):
            pw = psB.tile([80, HG], f32, tag="psc")
            nc.tensor.matmul(pw[:], wz[0:80, 0:80], wz[0:80, :],
                             start=True, stop=True)

        # ---------------- stage A: per 128-graph block ----------------
        ptbig = psT.tile([128, 512], bf16, tag="pt", name="ptbig")
        tps = []           # transpose psum regions in flight: (block, q, width)
        for b in range(NB):
            xe = xe_tiles[b]
            xv = xe[:, 0:294].rearrange("p (r c ch) -> p r c ch", r=7, c=7, ch=6)
            xC = xv[:, 1:6, 1:6, :]
            xW, xE = xv[:, 1:6, 0:5, :], xv[:, 1:6, 2:7, :]
            xN, xS = xv[:, 0:5, 1:6, :], xv[:, 2:7, 1:6, :]

            def attr(d):
                v = xe[:, 294 + d * 25:294 + (d + 1) * 25]
                v = v.rearrange("p (r c) -> p r c", r=5, c=5)
                return v.unsqueeze(3).broadcast_to([128, 5, 5, 6])

            aW, aE, aN, aS = attr(0), attr(1), attr(2), attr(3)

            zx = work.tile([128, 320], bf16, tag="zx")
            nc.gpsimd.memset(
                zx[:].rearrange("p (r k) -> p r k", r=5, k=64)[:, :, 60:64], 0.0)
            zxv = zx[:].rearrange("p (r k) -> p r k", r=5, k=64)[:, :, 0:60] \
                       .rearrange("p r (c ch) -> p r c ch", c=5, ch=12)
            zxz, zxx = zxv[:, :, :, 0:6], zxv[:, :, :, 6:12]

            def wtile(tag):
                t = work.tile([128, 150], bf16, tag=tag, name=tag)
                return t[:].rearrange("p (r c ch) -> p r c ch", r=5, c=5, ch=6)

            t1v, t2v = wtile("t1"), wtile("t2")
            t3v, t4v = wtile("t3"), wtile("t4")
            # z = aW*xW + aE*xE + aN*xN + aS*xS; mults split Pool/DVE,
            # adds on DVE (bf16 2x), center-x copy on Pool
            nc.gpsimd.tensor_tensor(t1v, xW, aW, mul)
            nc.gpsimd.tensor_tensor(t2v, xE, aE, mul)
            nc.vector.tensor_tensor(t3v, xN, aN, mul)
            nc.vector.tensor_tensor(t4v, xS, aS, mul)
            nc.gpsimd.tensor_copy(zxx, xC)
            nc.vector.tensor_tensor(t1v, t1v, t2v, add)
            nc.gpsimd.tensor_tensor(t3v, t3v, t4v, add)
            nc.vector.tensor_tensor(zxz, t1v, t3v, add)

            # transpose to feature-major (features on partitions); the four
            # 128-wide regions of ptbig act as independent psum slots
            for q in range(3):
                w = min(128, 320 - q * 128)
                k = (b * 3 + q) % 4
                nc.tensor.matmul(ptbig[0:w, k * 128:k * 128 + 128],
                                 zx[:, q * 128:q * 128 + w],
                                 ident_t[:], is_transpose=True,
                                 start=True, stop=True)
                tps.append((b, q, w))

            # drain finished transposes, alternating Pool/DVE
            while tps:
                bb, q, w = tps.pop(0)
                k = (bb * 3 + q) % 4
                dst = ZXT[q][0:w, bb * 128:(bb + 1) * 128]
                src_ = ptbig[0:w, k * 128:k * 128 + 128]
                if (bb * 3 + q) % 2 == 0:
                    nc.gpsimd.tensor_copy(dst, src_)
                else:
                    nc.vector.tensor_copy(dst, src_)

        # -------- stage B + conv stack, two pipelined graph half-streams
        def drain(dst, ps, bias_ap, eng):
            if eng == "act":
                nc.scalar.activation(dst, ps, relu, bias=bias_ap)
            elif eng == "dve":
                nc.vector.tensor_scalar(dst, ps, bias_ap, 0.0, add, max_)
            else:
                nc.gpsimd.tensor_scalar(dst, ps, bias_ap, 0.0, add, max_)

        SB_ENG = ["pool", "dve", "pool", "dve", "pool"]
        CV_ENG = ["pool", "dve", "act", "pool", "act"]

        def stage_b(h):
            gs = slice(h * HG, (h + 1) * HG)
            for r in range(5):
                t, off = zxt_row(r)
                ps = psB.tile([80, HG], f32, tag="psc")
                nc.tensor.matmul(ps[:], bd0_t[off:off + 60, 0:80],
                                 t[off:off + 60, gs], start=True, stop=True)
                drain(H[0][r][:, gs], ps[:], cb_t[0:80, 5:6], SB_ENG[r])

        def conv_layer(l, h):
            gs = slice(h * HG, (h + 1) * HG)
            for r in range(5):
                ps = psB.tile([80, HG], f32, tag="psc")
                dys = [dy for dy in (-1, 0, 1) if 0 <= r + dy <= 4]
                for i, dy in enumerate(dys):
                    nc.tensor.matmul(ps[:], convT(l, dy), H[l][r + dy][:, gs],
                                     start=(i == 0), stop=(i == len(dys) - 1))
                drain(H[l + 1][r][:, gs], ps[:], cb_t[0:80, l:l + 1], CV_ENG[r])

        def last_layer(h):
            gs = slice(h * HG, (h + 1) * HG)
            p0 = psL.tile([90, HG], f32, tag="p90")
            p1 = psL.tile([60, HG], f32, tag="p60")
            seg_first = {0: True, 1: True}
            n_seg = {0: 4, 1: 3}
            cnt = {0: 0, 1: 0}
            for r_in, seg in L5_PASSES:
                ps = p0 if seg == 0 else p1
                cnt[seg] += 1
                nc.tensor.matmul(ps[:], l5T(r_in, seg), H[5][r_in][:, gs],
                                 start=seg_first[seg], stop=(cnt[seg] == n_seg[seg]))
                seg_first[seg] = False
            # drain with last-layer bias (no relu): Pool / DVE split
            if h == 0:
                nc.vector.tensor_scalar(O0[:, gs], p0[:], cb_t[0:90, 6:7],
                                        None, add)
                nc.gpsimd.tensor_scalar(O1[:, gs], p1[:], cb_t[0:60, 7:8],
                                        None, add)
                nc.scalar.dma_start(out_ap[0:90, gs], O0[:, gs])
                nc.scalar.dma_start(out_ap[90:150, gs], O1[:, gs])
            else:
                nc.vector.tensor_scalar(O0[:, gs], p0[:], cb_t[0:90, 6:7],
                                        None, add)
                nc.gpsimd.tensor_scalar(O1[:, gs], p1[:], cb_t[0:60, 7:8],
                                        None, add)
                nc.scalar.dma_start(out_ap[0:90, gs], O0[:, gs])
                nc.sync.dma_start(out_ap[90:150, gs], O1[:, gs])

        stage_b(0)
        conv_layer(0, 0)
        stage_b(1)
        for l in range(5):
            if l > 0:
                conv_layer(l, 0)
            if l == 4:
                last_layer(0)
            conv_layer(l, 1)
        last_layer(1)


def _legalize_single_wait(nc):
    """This toolchain's walrus allows at most ONE sync wait per instruction
    (TPB_EVENTS has a single wait slot). Tile's sem assignment can emit
    several; hoist all but one onto same-engine NoOps inserted just before."""
    from concourse import mybir

    for fn in nc.m.functions:
        for blk in fn.blocks:
            insts = list(blk.instructions)
            out = []
            changed = False
            for inst in insts:
                si = getattr(inst, "sync_info", None)
                waits = list(si.on_wait) if si is not None and si.on_wait else []
                if len(waits) > 1:
                    for w in waits[:-1]:
                        nop = mybir.InstNoOp(
                            name=nc.get_next_instruction_name(), ins=[], outs=[])
                        nop.engine = inst.engine
                        nop.sync_info = mybir.SyncInfo(on_wait=[w], on_update=[])
                        nc.register_instruction(nop)
                        out.append(nop)
                    si.on_wait = [waits[-1]]
                    changed = True
                out.append(inst)
            if changed:
                blk.instructions[:] = out


_PROGRAM_CACHE = {}


def _get_program():
    if "p" in _PROGRAM_CACHE:
        return _PROGRAM_CACHE["p"]
    import concourse.bass as bass
    import concourse.tile as tile
    from concourse import mybir

    f32 = mybir.dt.float32
    bf16 = mybir.dt.bfloat16
    nc = bass.Bass()
    xes_t = nc.declare_dram_parameter("xes", [128, NB * XE_COLS], bf16,
                                      isOutput=False)
    wcr_t = nc.declare_dram_parameter("wcr", [128, WCOLS], bf16, isOutput=False)
    cbf_t = nc.declare_dram_parameter("cbf", [128, 8], f32, isOutput=False)
    # rows: out feature f = ro*30 + c*6 + ch  (5x5x6 row-major crop)
    out_t = nc.declare_dram_parameter("out", [150, GPC], f32, isOutput=True)
    with tile.TileContext(nc) as tc:
        _build(tc, out_t[:], xes_t[:], wcr_t[:], cbf_t[:])
    _legalize_single_wait(nc)
    _PROGRAM_CACHE["p"] = nc
    return nc


def _prep_core_inputs(inputs):
    """Shared with test.py: returns (xes per-core list, wcr, cbf)."""
    x = np.ascontiguousarray(np.asarray(inputs["x"], np.float32))
    ea = np.ascontiguousarray(np.asarray(inputs["edge_attr"], np.float32))
    cws = [np.asarray(inputs[f"cw{i}"], np.float32) for i in range(6)]
    cbs = [np.asarray(inputs[f"cb{i}"], np.float32) for i in range(6)]
    wcr, cbf = _fold_weights(np.asarray(inputs["w1"], np.float32),
                             np.asarray(inputs["b1"], np.float32),
                             np.asarray(inputs["w2"], np.float32),
                             np.asarray(inputs["b2"], np.float32),
                             np.asarray(inputs["w_root"], np.float32),
                             np.asarray(inputs["bias"], np.float32), cws, cbs)
    xe = _pack_xe(x, ea)
    xes = []
    for c in range(NCORES):
        xec = xe[c * GPC:(c + 1) * GPC].reshape(NB, 128, XE_COLS)
        xes.append(np.ascontiguousarray(
            np.concatenate([xec[b] for b in range(NB)], axis=1)))
    return xes, wcr, cbf


def _unshard(od):
    """[150, GPC] row-major (ro, c, ch) -> [GPC, 150] channel-major."""
    return od.reshape(5, 5, 6, GPC).transpose(3, 2, 0, 1).reshape(GPC, 150)


# ---------------------------------------------------------------------------
# numpy fallback (only if structural assumptions fail)
# ---------------------------------------------------------------------------

def _numpy_reference(x, edge_index, edge_attr, w1, b1, w2, b2, w_root, bias,
                     cws, cbs):
    N = x.shape[0]
    B = N // NPG  # shadow module constant: stay correct for any batch
    src, dst = np.asarray(edge_index[0]), np.asarray(edge_index[1])
    h = np.maximum(edge_attr @ w1 + b1, 0.0)
    w_e = (h @ w2 + b2).reshape(-1, 6, 16)
    msg = np.einsum('ei,eio->eo', x[src], w_e)
    agg = np.zeros((N, 16), np.float32)
    np.add.at(agg, dst, msg)
    out = np.maximum(agg + x @ w_root + bias, 0.0)
    img = out.reshape(B, NPG, 16).transpose(0, 2, 1).reshape(B, 16, 9, 9)
    img = img[:, :, 2:7, 2:7].transpose(0, 2, 3, 1)          # NHWC
    for i in range(6):
        cw, cb = cws[i], cbs[i]
        O = cw.shape[0]
        o = np.zeros((B, 5, 5, O), np.float32)
        for dy in (-1, 0, 1):
            for dx in (-1, 0, 1):
                ys, ye = max(0, -dy), min(5, 5 - dy)
                xs, xe = max(0, -dx), min(5, 5 - dx)
                o[:, ys:ye, xs:xe, :] += img[:, ys + dy:ye + dy, xs + dx:xe + dx, :] \
                    @ cw[:, :, dy + 1, dx + 1].T
        o += cb
        img = np.maximum(o, 0.0) if i < 5 else o
    return img.transpose(0, 3, 1, 2).reshape(B, -1).astype(np.float32)


_GRID_OK_CACHE = {}


def _grid_ok(edge_index):
    key = id(edge_index)
    if key in _GRID_OK_CACHE:
        return _GRID_OK_CACHE[key]
    idx = np.arange(NPG).reshape(9, 9)
    src0 = np.concatenate([idx[:, :-1].ravel(), idx[:, 1:].ravel(),
                           idx[:-1, :].ravel(), idx[1:, :].ravel()])
    dst0 = np.concatenate([idx[:, 1:].ravel(), idx[:, :-1].ravel(),
                           idx[1:, :].ravel(), idx[:-1, :].ravel()])
    off = (np.arange(B, dtype=np.int64) * NPG)[:, None]
    ei = np.asarray(edge_index)
    ok = (ei.shape == (2, B * EPG)
          and np.array_equal(ei[0].reshape(B, EPG), src0[None, :] + off)
          and np.array_equal(ei[1].reshape(B, EPG), dst0[None, :] + off))
    _GRID_OK_CACHE[key] = ok
    return ok


def kernel(x, edge_index, edge_attr, w1, b1, w2, b2, w_root, bias,
           cw0, cb0, cw1, cb1, cw2, cb2, cw3, cb3, cw4, cb4, cw5, cb5):
    x = np.ascontiguousarray(np.asarray(x, np.float32))
    edge_attr = np.ascontiguousarray(np.asarray(edge_attr, np.float32))
    w1, b1 = np.asarray(w1, np.float32), np.asarray(b1, np.float32)
    w2, b2 = np.asarray(w2, np.float32), np.asarray(b2, np.float32)
    w_root, bias = np.asarray(w_root, np.float32), np.asarray(bias, np.float32)
    cws = [np.asarray(c, np.float32) for c in (cw0, cw1, cw2, cw3, cw4, cw5)]
    cbs = [np.asarray(c, np.float32) for c in (cb0, cb1, cb2, cb3, cb4, cb5)]

    structural_ok = (
        x.shape == (B * NPG, 6)
        and edge_attr.shape == (B * EPG, 1)
        and np.all(b1 == 0.0)
        and np.all(b2 == 0.0)
        and float(edge_attr.min()) >= 0.0
        and _grid_ok(edge_index)
    )
    if not structural_ok:
        return _numpy_reference(x, edge_index, edge_attr, w1, b1, w2, b2,
                                w_root, bias, cws, cbs)

    from concourse.bass_utils import run_bass_kernel_spmd

    nc = _get_program()
    inputs = dict(x=x, edge_attr=edge_attr, w1=w1, b1=b1, w2=w2, b2=b2,
                  w_root=w_root, bias=bias)
    for i in range(6):
        inputs[f"cw{i}"] = cws[i]
        inputs[f"cb{i}"] = cbs[i]
    xes, wcr, cbf = _prep_core_inputs(inputs)
    in_maps = [{"xes": xes[c], "wcr": wcr, "cbf": cbf} for c in range(NCORES)]
    trace = os.environ.get("BASSK_TRACE", "0") == "1"
    if trace:
        import importlib.util
        if importlib.util.find_spec("antenv.axon_hooks") is None:
            trace = False
    res = run_bass_kernel_spmd(nc, in_maps, list(range(NCORES)), trace=trace)
    global LAST_EXEC_TIME_NS
    LAST_EXEC_TIME_NS = getattr(res, "exec_time_ns", None)
    outs = [_unshard(np.asarray(res.results[c]["out"], np.float32))
            for c in range(NCORES)]
    return np.ascontiguousarray(np.concatenate(outs, axis=0), np.float32)


LAST_EXEC_TIME_NS = None
